# revision 32
# baseline (speedup 1.0000x reference)
"""Trainium2 Bass kernel for a 2-layer encoder-decoder LSTM.

Problem: x [512, 256, 1] -> encoder 2-layer LSTM (H=512) -> autoregressive
decoder (64 steps, head feedback) -> out [512, 64].

Strategy: data-parallel across 8 NeuronCores (batch 512 -> 64 per core), all
weights replicated and SBUF-resident.  Per core each timestep needs 3
matmuls of [64,512] @ [512,2048], run "activation-stationary" (lhsT = h.T
chunk [128,64], moving = W.T [128,512] slices).  Each "pair" step computes
layer-1 @ t together with layer-0 @ t+1 so the PE always has a deep stream
of independent work.

Two dtype modes (LSTM_MMDT):
 - bfloat16: PE column tiling packs the two cells onto separate column
   groups (layer-1 -> PSUM partitions 0..63, layer-0 -> 64..127) which run
   concurrently, and the activation/vector engines process both cells
   stacked [128, *] at full width.
 - float32r: tf32-like full-width mode (the PE uses both columns of each
   column pair, so no column tiling is possible); the two cells use
   separate PSUM tiles on partitions 0..63.

Biases and the scalar input term are folded into extra K=1/K=2 matmul
accumulation passes.  h is re-transposed each step with PE transpose; the
transposed h.T doubles as the moving operand of the decoder head matmul.
"""

import os
import sys
import time

import numpy as np

B_FULL, T, H, HORIZON = 512, 256, 512, 64
NCORES = 8
B = B_FULL // NCORES          # 64 batch rows per core
G = 4 * H                     # 2048 gate columns
KC = H // 128                 # 4 contraction chunks
NCH = G // 512                # 4 output chunks of 512 (one PSUM bank each)

# gate chunk indices (PyTorch order: i, f, g, o)
I_SL, F_SL, G_SL, O_SL = 0, 1, 2, 3

_CACHE = {}
LAST_EXEC_NS = None
LAST_RESULTS = None


def _build(n_enc=T, n_dec=HORIZON, mm_dt="float32r"):
    """Build the Bass module (single SPMD program, run on all 8 cores)."""
    from contextlib import ExitStack

    import concourse.mybir as mybir
    import concourse.tile as tile
    from concourse import bacc
    from concourse.masks import make_identity

    dt = mybir.dt
    MDT = getattr(dt, mm_dt)
    F32 = dt.float32
    AF = mybir.ActivationFunctionType
    NT = n_enc + n_dec            # total timesteps
    PAIRED = mm_dt != "float32r"  # col-tiled two-group mode

    nc = bacc.Bacc("TRN2", target_bir_lowering=False, debug=False)

    # ---------------- DRAM parameters (per-core views) ----------------
    xaug = nc.declare_dram_parameter("xaug", [2, (n_enc + 1) * B], MDT, isOutput=False)
    wt = {}
    for nm in ("e0", "e1i", "e1h", "d0", "d1i", "d1h"):
        wt[nm] = nc.declare_dram_parameter(f"wt_{nm}", [128, KC, G], MDT, isOutput=False)
    rows_e0 = nc.declare_dram_parameter("rows_e0", [2, G], MDT, isOutput=False)
    rows_e1 = nc.declare_dram_parameter("rows_e1", [1, G], MDT, isOutput=False)
    rows_d0 = nc.declare_dram_parameter("rows_d0", [2, G], MDT, isOutput=False)
    rows_d1 = nc.declare_dram_parameter("rows_d1", [1, G], MDT, isOutput=False)
    headt_d = nc.declare_dram_parameter("headt", [128, KC], MDT, isOutput=False)
    headb_d = nc.declare_dram_parameter("headb", [1, B], MDT, isOutput=False)
    zeros_d = nc.declare_dram_parameter("zeros", [128, KC * B], MDT, isOutput=False)
    outT = nc.declare_dram_parameter("outT", [1, n_dec * B], F32, isOutput=True)

    with ExitStack() as ctx:
        tc = ctx.enter_context(tile.TileContext(nc))
        wpool = ctx.enter_context(tc.tile_pool(name="w", bufs=1))
        consts = ctx.enter_context(tc.tile_pool(name="consts", bufs=1))
        states = ctx.enter_context(tc.tile_pool(name="states", bufs=2))
        # loop-carried tags (fused state, c) need capacity >= 3 across the
        # For_i back edge and a buf count dividing the 8-pair body
        statesL = ctx.enter_context(tc.tile_pool(name="statesL", bufs=4))
        xstage = ctx.enter_context(tc.tile_pool(name="xstage", bufs=1))
        acts = ctx.enter_context(tc.tile_pool(name="acts", bufs=2))
        gpool = ctx.enter_context(
            tc.tile_pool(name="gp", bufs=4, space="PSUM"))
        tpool = ctx.enter_context(tc.tile_pool(name="tp", bufs=2, space="PSUM"))

        # ---------------- constants ----------------
        ident = consts.tile([128, 128], F32, tag="ident")
        make_identity(nc, ident)
        identM = consts.tile([128, 128], MDT, tag="identM")
        make_identity(nc, identM)
        xall = consts.tile([2, (n_enc + 1) * B], MDT, tag="xall")
        nc.sync.dma_start(out=xall, in_=xaug[:, :])
        ones64 = consts.tile([1, B], MDT, tag="ones64")
        nc.sync.dma_start(out=ones64, in_=xaug[1:2, 0:B])
        headt = consts.tile([128, KC], MDT, tag="headt")
        nc.sync.dma_start(out=headt, in_=headt_d[:, :])
        headb = consts.tile([1, B], MDT, tag="headb")
        nc.sync.dma_start(out=headb, in_=headb_d[:, :])
        dec_stage = consts.tile([2, B], MDT, tag="dec_stage")
        # (x_last, ones): row 1 stays 1.0 forever; row 0 overwritten per step
        nc.vector.tensor_copy(dec_stage, xall[:, n_enc * B:(n_enc + 1) * B])
        out_acc = consts.tile([1, n_dec * B], F32, tag="out_acc")

        # weight tiles: encoder set now, decoder set later (same tags)
        def load_wset(phase):
            p = "e" if phase == 0 else "d"
            w0 = wpool.tile([128, KC, G], MDT, tag="w0")
            nc.sync.dma_start(out=w0, in_=wt[p + "0"][:, :, :])
            w1i = wpool.tile([128, KC, G], MDT, tag="w1i")
            nc.sync.dma_start(out=w1i, in_=wt[p + "1i"][:, :, :])
            w1h = wpool.tile([128, KC, G], MDT, tag="w1h")
            nc.sync.dma_start(out=w1h, in_=wt[p + "1h"][:, :, :])
            r0 = wpool.tile([2, G], MDT, tag="rows0")
            nc.sync.dma_start(out=r0, in_=(rows_e0 if phase == 0 else rows_d0)[:, :])
            r1 = wpool.tile([1, G], MDT, tag="rows1")
            nc.sync.dma_start(out=r1, in_=(rows_e1 if phase == 0 else rows_d1)[:, :])
            return dict(w0=w0, w1i=w1i, w1h=w1h, r0=r0, r1=r1)

        enc_w = load_wset(0)
        dec_w = None

        TOP = slice(0, 64)
        BOT = slice(64, 128)

        # ---------------- initial state ----------------
        # state accessors: h0ap(k)/h1ap(k) -> [128, B] lhsT chunk views
        h1T0 = states.tile([128, KC * B], MDT, tag="h1T")
        nc.sync.dma_start(out=h1T0, in_=zeros_d[:, :])
        h0ap = None                      # set by prologue
        h1ap = lambda k, t=h1T0: t[:, k * B:(k + 1) * B]
        if PAIRED:
            c_prev = statesL.tile([128, 512], F32, tag="c")
            nc.vector.memset(c_prev, 0.0)
            c1_prev = c0_prev = None
        else:
            c1_prev = states.tile([64, 512], F32, tag="c1")
            nc.vector.memset(c1_prev, 0.0)
            c0_prev = states.tile([64, 512], F32, tag="c0")
            nc.vector.memset(c0_prev, 0.0)
            c_prev = None

        def alloc_tset(sfx):
            return dict(
                ifsb=acts.tile([128, 1024], F32, tag="ifsb" + sfx, name="ifsb" + sfx),
                gsb=acts.tile([128, 512], F32, tag="gsb" + sfx, name="gsb" + sfx),
                osb=acts.tile([128, 512], F32, tag="osb" + sfx, name="osb" + sfx),
                t1=acts.tile([128, 512], F32, tag="t1" + sfx, name="t1" + sfx),
                t2=acts.tile([128, 512], F32, tag="t2" + sfx, name="t2" + sfx),
                tcsb=acts.tile([128, 512], F32, tag="tcsb" + sfx, name="tcsb" + sfx),
                # MDT: h is fp16 in the state anyway; fp16 input makes the PE
                # transpose run at 1 cycle/row instead of 2
                hsb=acts.tile([128, 512], MDT, tag="hsb" + sfx, name="hsb" + sfx),
            )

        def emit_cell(gps, gsl, ts, sl, c_prev_ap, c_new_ap):
            """One LSTM cell's activations + state update.
            gps: 4 psum chunk tiles; gsl: partition slice in psum;
            ts: act tile set; sl: partition slice in act tiles."""
            nc.scalar.activation(ts["ifsb"][sl, 0:512], gps[I_SL][gsl, :], AF.Sigmoid)
            nc.scalar.activation(ts["ifsb"][sl, 512:1024], gps[F_SL][gsl, :], AF.Sigmoid)
            nc.scalar.activation(ts["gsb"][sl, :], gps[G_SL][gsl, :], AF.Tanh)
            nc.vector.tensor_mul(ts["t1"][sl, :], ts["ifsb"][sl, 512:1024], c_prev_ap)
            nc.vector.tensor_mul(ts["t2"][sl, :], ts["ifsb"][sl, 0:512], ts["gsb"][sl, :])
            nc.vector.tensor_add(c_new_ap, ts["t1"][sl, :], ts["t2"][sl, :])
            # tanh(c) before sigmoid(o) in ACT program order: o depends on the
            # last-finishing gate chunk, tanh(c) only on i/f/g
            nc.scalar.activation(ts["tcsb"][sl, :], c_new_ap, AF.Tanh)
            nc.scalar.activation(ts["osb"][sl, :], gps[O_SL][gsl, :], AF.Sigmoid)
            nc.vector.tensor_mul(ts["hsb"][sl, :], ts["osb"][sl, :], ts["tcsb"][sl, :])

        def emit_transpose(h_src, ident_blk, state_tag):
            """h_src [64, 512] -> new [128, KC*B] transposed state tile."""
            tp = tpool.tile([128, KC * B], MDT, tag="tp", name="tp" + state_tag)
            for k in range(KC):
                nc.tensor.transpose(tp[:, k * B:(k + 1) * B],
                                    h_src[:, k * 128:(k + 1) * 128], ident_blk)
            new = states.tile([128, KC * B], MDT, tag=state_tag, name=state_tag)
            nc.vector.tensor_copy(new, tp)
            return new

        def emit_transpose_fused(h_src):
            """h_src [128, 512] (both cells) -> [128, 512] fused state tile.
            Chunk k cols 0:64 = TOP cell h.T, cols 64:128 = BOT cell h.T."""
            tps = []
            for half in range(2):
                tp = tpool.tile([128, KC * B], F32, tag="tp", name=f"tpf{half}")
                for kk in range(2):
                    k = half * 2 + kk
                    nc.tensor.transpose(tp[:, kk * 128:(kk + 1) * 128],
                                        h_src[:, k * 128:(k + 1) * 128], ident)
                tps.append(tp)
            new = statesL.tile([128, 512], MDT, tag="hTf", name="hTf")
            nc.vector.tensor_copy(new[:, 0:256], tps[0])
            nc.vector.tensor_copy(new[:, 256:512], tps[1])
            return new

        def emit_pair(s, top, bottom, stage_override=None):
            """TOP: layer-1 cell @ time s.  BOTTOM: layer-0 cell @ time s+1."""
            nonlocal h0ap, h1ap, c_prev, c1_prev, c0_prev, dec_w
            u = s + 1  # bottom timestep
            if bottom and u >= n_enc and dec_w is None:
                dec_w = load_wset(1)
            wtop = enc_w if (top and s < n_enc) else dec_w
            wbot = enc_w if (bottom and u < n_enc) else dec_w
            feedback = bottom and u > n_enc  # bottom x comes from this pair's head

            mm_h0ap, mm_h1ap = h0ap, h1ap
            stage = None
            if bottom:
                if stage_override is not None:
                    stage = stage_override
                elif not feedback:  # encoder steps + first decoder step: resident
                    stage = xall[:, u * B:(u + 1) * B]
                else:
                    stage = dec_stage

            # ---------------- matmul passes ----------------
            if PAIRED:
                gps_t = gps_b = [gpool.tile([128, 512], F32, tag="gp", name=f"gp{j}")
                                 for j in range(NCH)]
                bot_gsl, bot_tpos = BOT, (0, 64)
            else:
                gps_t = [gpool.tile([64, 512], F32, tag="gp", name=f"gpt{j}")
                         for j in range(NCH)] if top else None
                gps_b = [gpool.tile([64, 512], F32, tag="gp", name=f"gpb{j}")
                         for j in range(NCH)] if bottom else None
                bot_gsl, bot_tpos = slice(0, 64), (0, 0)

            a_seq = []  # top: bias1, wih1 x16, whh1 x16
            b_seq = []  # bottom: whh0 x16 (+ xb x4 if not feedback)
            first_b = [True] * NCH
            if top:
                for j in range(NCH):
                    a_seq.append((gps_t[j][TOP, :], ones64[0:1, :],
                                  wtop["r1"][0:1, j * 512:(j + 1) * 512], True, False))
                for j in range(NCH):
                    for k in range(KC):
                        a_seq.append((gps_t[j][TOP, :], mm_h0ap(k),
                                      wtop["w1i"][:, k, j * 512:(j + 1) * 512], False, False))
                # chunk-contiguous: chunk j's gates complete in order i,f,g,o so
                # the ACT/cell chain starts while later chunks still accumulate
                for j in range(NCH):
                    for k in range(KC):
                        a_seq.append((gps_t[j][TOP, :], mm_h1ap(k),
                                      wtop["w1h"][:, k, j * 512:(j + 1) * 512],
                                      False, k == KC - 1))
            if bottom:
                for j in range(NCH):
                    if mm_h0ap is not None:
                        for k in range(KC):
                            b_seq.append((gps_b[j][bot_gsl, :], mm_h0ap(k),
                                          wbot["w0"][:, k, j * 512:(j + 1) * 512],
                                          first_b[j], False))
                            first_b[j] = False
                    if not feedback:
                        b_seq.append((gps_b[j][bot_gsl, :], stage[0:2, :],
                                      wbot["r0"][0:2, j * 512:(j + 1) * 512],
                                      first_b[j], True))
                        first_b[j] = False

            # emission order: bias passes, then bottom-dense 1:1 with top, then
            # the rest of top.  The bottom cell's matmuls finish mid-pair so its
            # h.T (needed by almost all of the next pair) is ready by pair end.
            # emit_mms(phase=0) emits through the end of the bottom stream (the
            # caller then emits the bottom cell + transposes so they land
            # mid-stream in the PE queue); emit_mms(phase=1) emits the rest.
            nbias = NCH if top else 0
            na, nb = len(a_seq), len(b_seq)
            order = [("a", x) for x in a_seq[:nbias]]
            ia, ib = nbias, 0
            if PAIRED:
                # 1:1 zip: both column groups advance at their own full rate
                # (starts are pc-monotone but execution is concurrent), so the
                # bottom group finishes at ~nb passes while A streams on
                while ib < nb:
                    order.append(("b", b_seq[ib])); ib += 1
                    if ia < na:
                        order.append(("a", a_seq[ia])); ia += 1
            else:
                order.extend(("b", x) for x in b_seq)  # bottom block first, dense
                ib = nb
            split0 = len(order)
            order.extend(("a", x) for x in a_seq[ia:])
            # phase boundaries: [0: bias+bottom][1: ~12 top passes][2: rest]
            split1 = min(split0 + 12, len(order))

            def emit_mms(phase):
                lo, hi = [(0, split0), (split0, split1), (split1, len(order))][phase]
                for grp, (out, lhsT, rhs, st, sp) in order[lo:hi]:
                    nc.tensor.matmul(out, lhsT, rhs, start=st, stop=sp,
                                     tile_position=(0, 0) if grp == "a" else bot_tpos,
                                     skip_group_check=True)

            # ---------------- activations + cell + transpose ----------------
            if PAIRED:
                ts_t = ts_b = alloc_tset("")
                c_new = statesL.tile([128, 512], F32, tag="c", name="c")
                if not (top and bottom):
                    nc.vector.memset(c_new[BOT if top else TOP, :], 0.0)
                cell_top = lambda: emit_cell(gps_t, TOP, ts_t, TOP,
                                             c_prev[TOP, :], c_new[TOP, :])
                cell_bot = lambda: emit_cell(gps_b, BOT, ts_b, BOT,
                                             c_prev[BOT, :], c_new[BOT, :])
                top_h = lambda: ts_t["hsb"][TOP, :]
                bot_h = lambda: ts_b["hsb"][BOT, :]
                bot_ident = identM[64:128, 64:128]
            else:
                ts_t = alloc_tset("t") if top else None
                ts_b = alloc_tset("b") if bottom else None
                c1_new = (states.tile([64, 512], F32, tag="c1", name="c1")
                          if top else None)
                c0_new = (states.tile([64, 512], F32, tag="c0", name="c0")
                          if bottom else None)
                cell_top = lambda: emit_cell(gps_t, slice(0, 64), ts_t, TOP,
                                             c1_prev[:, :], c1_new[:, :])
                cell_bot = lambda: emit_cell(gps_b, slice(0, 64), ts_b, TOP,
                                             c0_prev[:, :], c0_new[:, :])
                top_h = lambda: ts_t["hsb"][TOP, :]
                bot_h = lambda: ts_b["hsb"][TOP, :]
                bot_ident = identM[0:64, 0:64]

            def head():
                d = s - n_enc
                hd = tpool.tile([128, KC * B], F32, tag="hd", name="hd")[0:1, 0:B]
                nc.tensor.matmul(hd, ones64[0:1, 0:1], headb[0:1, :],
                                 start=True, stop=False)
                for k in range(KC):
                    nc.tensor.matmul(hd, headt[:, k:k + 1], h1ap(k),
                                     start=False, stop=k == KC - 1)
                if d + 1 < n_dec:
                    nc.vector.tensor_copy(dec_stage[0:1, :], hd)
                nc.vector.tensor_copy(out_acc[0:1, d * B:(d + 1) * B], hd)

            if feedback:
                # decoder: top cell -> head -> bottom x pass -> bottom cell
                emit_mms(0)
                emit_mms(1)
                emit_mms(2)
                cell_top()
                t1T = emit_transpose(top_h(), identM[0:64, 0:64], "h1T")
                h1ap = lambda k, t=t1T: t[:, k * B:(k + 1) * B]
                head()
                for j in range(NCH):
                    nc.tensor.matmul(gps_b[j][bot_gsl, :], stage[0:2, :],
                                     wbot["r0"][0:2, j * 512:(j + 1) * 512],
                                     start=first_b[j], stop=True,
                                     tile_position=bot_tpos, skip_group_check=True)
                cell_bot()
                t0T = emit_transpose(bot_h(), bot_ident, "h0T")
                h0ap = lambda k, t=t0T: t[:, k * B:(k + 1) * B]
            elif PAIRED and top and bottom:
                # full-width path: one cell chain for both cells; the o-gate /
                # h / transpose tail runs per 128-col chunk so transposes and
                # the state copies start as soon as each chunk is ready
                emit_mms(0)
                emit_mms(1)
                emit_mms(2)
                ts = ts_t
                fl = slice(0, 128)
                nc.scalar.activation(ts["ifsb"][fl, 0:512], gps_t[I_SL][fl, :], AF.Sigmoid)
                nc.scalar.activation(ts["ifsb"][fl, 512:1024], gps_t[F_SL][fl, :], AF.Sigmoid)
                nc.scalar.activation(ts["gsb"][fl, :], gps_t[G_SL][fl, :], AF.Tanh)
                nc.vector.tensor_mul(ts["t1"][fl, :], ts["ifsb"][fl, 512:1024], c_prev[:, :])
                nc.vector.tensor_mul(ts["t2"][fl, :], ts["ifsb"][fl, 0:512], ts["gsb"][fl, :])
                nc.vector.tensor_add(c_new[:, :], ts["t1"][fl, :], ts["t2"][fl, :])
                nc.scalar.activation(ts["tcsb"][fl, :], c_new[:, :], AF.Tanh)
                tps = [tpool.tile([128, KC * B], MDT, tag="tp", name=f"tpf{h2}")
                       for h2 in range(2)]
                fused = statesL.tile([128, 512], MDT, tag="hTf", name="hTf")
                for k in range(KC):
                    cs = slice(k * 128, (k + 1) * 128)
                    nc.scalar.activation(ts["osb"][fl, cs], gps_t[O_SL][fl, cs], AF.Sigmoid)
                    nc.vector.tensor_mul(ts["hsb"][fl, cs], ts["osb"][fl, cs],
                                         ts["tcsb"][fl, cs])
                    nc.tensor.transpose(tps[k // 2][:, (k % 2) * 128:(k % 2 + 1) * 128],
                                        ts["hsb"][:, cs], identM)
                    if k % 2 == 1:
                        nc.vector.tensor_copy(fused[:, (k // 2) * 256:(k // 2 + 1) * 256],
                                              tps[k // 2])
                h1ap = lambda k, t=fused: t[:, k * 128:k * 128 + 64]
                h0ap = lambda k, t=fused: t[:, k * 128 + 64:(k + 1) * 128]
            else:
                # bottom first; its transposes go into the PE stream a dozen
                # passes later so the PE reaches them just as the bottom
                # cell's ACT/DVE chain finishes (no head-of-line stall)
                emit_mms(0)
                if bottom:
                    cell_bot()
                emit_mms(1)
                if bottom:
                    t0T = emit_transpose(bot_h(), bot_ident, "h0T")
                    h0ap = lambda k, t=t0T: t[:, k * B:(k + 1) * B]
                emit_mms(2)
                if top:
                    cell_top()
                    t1T = emit_transpose(top_h(), identM[0:64, 0:64], "h1T")
                    h1ap = lambda k, t=t1T: t[:, k * B:(k + 1) * B]
                    if s >= n_enc:
                        head()

            if PAIRED:
                c_prev = c_new
            else:
                if top:
                    c1_prev = c1_new
                if bottom:
                    c0_prev = c0_new

        # prologue: layer-0 @ t=0 alone, then first fw pair unrolled
        emit_pair(-1, top=False, bottom=True)
        BODY = 8
        # encoder fw pairs s=1..n_enc-2 go through a hardware loop (8-pair
        # body); everything s-dependent inside is either constant for these
        # pairs (enc weights, no head) or overridden (x stage).
        nloop = max(0, (n_enc - 2) // BODY) if PAIRED else 0
        s_after_loop = 1 + nloop * BODY
        emit_pair(0, top=True, bottom=True)
        if nloop > 0:
            from concourse.bass import ds
            with tc.For_i(0, nloop, 1) as iv:
                stage_body = xstage.tile([2, BODY * B], MDT, tag="stage_body")
                nc.sync.dma_start(
                    out=stage_body,
                    in_=xaug[:, ds(2 * B + iv * (BODY * B), BODY * B)])
                for p in range(BODY):
                    emit_pair(1 + p, top=True, bottom=True,
                              stage_override=stage_body[:, p * B:(p + 1) * B])
        for s in range(s_after_loop, NT - 1):
            emit_pair(s, top=True, bottom=True)
        emit_pair(NT - 1, top=True, bottom=False)
        nc.sync.dma_start(out=outT[:, :], in_=out_acc)

    nc.compile()
    return nc


def _build2(n_enc=T, n_dec=HORIZON, mm_dt="float16"):
    """v2: paired col-groups, half-width (N=256) staggered streams.

    Per pair s: group A (PE cols 0:63, PSUM rows 0:64) = layer-1 @ s;
    group B (cols 64:127, rows 64:128) = layer-0 @ s+1.  Gate chunks are
    emitted in order (g, i, f, o) and each gate's 512 columns are split
    into two 256-wide halves streamed back to back, so half 0's cell
    chain (tanh(g) -> c -> tanh(c) -> o*tc -> transpose -> copy01)
    completes BEFORE the stream ends and the next pair's k0/k1 passes
    never stall.  Decoder feedback is folded into a precomputed rank-1
    matrix W' = Wih0 @ headW so decoder pairs stream like encoder pairs;
    the head output y_s is computed off the critical path.
    """
    from contextlib import ExitStack

    import concourse.mybir as mybir
    import concourse.tile as tile
    from concourse import bacc
    from concourse.masks import make_identity

    dt = mybir.dt
    MDT = getattr(dt, mm_dt)
    F32 = dt.float32
    AF = mybir.ActivationFunctionType
    NT = n_enc + n_dec
    HB = 256                      # half width in gate columns
    CO = (G_SL, I_SL, F_SL, O_SL)  # chunk processing order

    teacher = os.environ.get("LSTM_TEACHER", "") != ""

    nc = bacc.Bacc("TRN2", target_bir_lowering=False, debug=False)

    nxa = (n_enc + n_dec) if teacher else (n_enc + 1)
    xaug = nc.declare_dram_parameter("xaug", [2, nxa * B], MDT, isOutput=False)
    wt = {}
    for nm in ("e0", "e1i", "e1h", "d0", "d1i", "d1h", "dW"):
        wt[nm] = nc.declare_dram_parameter(f"wt_{nm}", [128, KC, G], MDT, isOutput=False)
    rows_e0 = nc.declare_dram_parameter("rows_e0", [2, G], MDT, isOutput=False)
    rows_e1 = nc.declare_dram_parameter("rows_e1", [1, G], MDT, isOutput=False)
    rows_d0 = nc.declare_dram_parameter("rows_d0", [2, G], MDT, isOutput=False)
    rows_d1 = nc.declare_dram_parameter("rows_d1", [1, G], MDT, isOutput=False)
    rows_dP = nc.declare_dram_parameter("rows_dP", [1, G], MDT, isOutput=False)
    headt_d = nc.declare_dram_parameter("headt", [128, KC], MDT, isOutput=False)
    headb_d = nc.declare_dram_parameter("headb", [1, B], MDT, isOutput=False)
    outT = nc.declare_dram_parameter("outT", [1, n_dec * B], F32, isOutput=True)

    with ExitStack() as ctx:
        tc = ctx.enter_context(tile.TileContext(nc))
        wpool = ctx.enter_context(tc.tile_pool(name="w", bufs=1))
        consts = ctx.enter_context(tc.tile_pool(name="consts", bufs=1))
        statesL = ctx.enter_context(tc.tile_pool(name="statesL", bufs=4))
        xstage = ctx.enter_context(tc.tile_pool(name="xstage", bufs=1))
        acts = ctx.enter_context(tc.tile_pool(name="acts", bufs=2))
        gpool = ctx.enter_context(tc.tile_pool(name="gp", bufs=4, space="PSUM"))
        tpool = ctx.enter_context(tc.tile_pool(name="tp", bufs=2, space="PSUM"))

        identM = consts.tile([128, 128], MDT, tag="identM")
        make_identity(nc, identM)
        xall = consts.tile([2, nxa * B], MDT, tag="xall")
        nc.sync.dma_start(out=xall, in_=xaug[:, :])
        ones64 = consts.tile([1, B], MDT, tag="ones64")
        nc.sync.dma_start(out=ones64, in_=xaug[1:2, 0:B])
        headt = consts.tile([128, KC], MDT, tag="headt")
        nc.sync.dma_start(out=headt, in_=headt_d[:, :])
        headb = consts.tile([1, B], MDT, tag="headb")
        nc.sync.dma_start(out=headb, in_=headb_d[:, :])
        dec_stage = consts.tile([2, B], MDT, tag="dec_stage")
        # (y, ones): row 1 stays 1.0 forever; row 0 overwritten per step
        nc.vector.tensor_copy(dec_stage, xall[:, n_enc * B:(n_enc + 1) * B])
        out_acc = consts.tile([1, n_dec * B], F32, tag="out_acc")

        def load_wset(phase):
            p = "e" if phase == 0 else "d"
            ws = {}
            for key, dram in (("w0", wt[p + "0"]), ("w1i", wt[p + "1i"]),
                              ("w1h", wt[p + "1h"])):
                t = wpool.tile([128, KC, G], MDT, tag=key)
                nc.sync.dma_start(out=t, in_=dram[:, :, :])
                ws[key] = t
            r0 = wpool.tile([2, G], MDT, tag="rows0")
            nc.sync.dma_start(out=r0, in_=(rows_e0 if phase == 0 else rows_d0)[:, :])
            ws["r0"] = r0
            r1 = wpool.tile([1, G], MDT, tag="rows1")
            nc.sync.dma_start(out=r1, in_=(rows_e1 if phase == 0 else rows_d1)[:, :])
            ws["r1"] = r1
            return ws

        enc_w = load_wset(0)
        dec_w = None

        TOP = slice(0, 64)
        BOT = slice(64, 128)
        A_POS, B_POS = (0, 0), (0, 64)

        # ---- loop state: fused h.T [128, 512] (chunk k cols k*128:k*128+64
        # = h1.T, +64:+128 = h0.T), c [128, 512] (rows 0:64 c1, 64:128 c0).
        pending_tail = []

        fused = statesL.tile([128, 512], MDT, tag="hTf", name="hTf0", bufs=8)
        nc.vector.memset(fused, 0.0)
        c_prev = statesL.tile([128, 512], F32, tag="c", name="c0")
        nc.vector.memset(c_prev, 0.0)

        def tset():
            return dict(
                ifsb=acts.tile([128, 1024], F32, tag="ifsb", name="ifsb"),
                gsb=acts.tile([128, 512], F32, tag="gsb", name="gsb"),
                osb=acts.tile([128, 512], F32, tag="osb", name="osb"),
                t1=acts.tile([128, 512], F32, tag="t1", name="t1"),
                t2=acts.tile([128, 512], F32, tag="t2", name="t2"),
                tcsb=acts.tile([128, 512], F32, tag="tcsb", name="tcsb"),
                hsb=acts.tile([128, 512], MDT, tag="hsb", name="hsb"),
            )

        def emit_pair(s, stage_override=None, can_defer=True):
            """Emit one pair.  s in [-1, NT-1]; top iff s>=0, bottom iff
            s<NT-1.  Returns nothing; updates fused/c_prev."""
            nonlocal fused, c_prev, dec_w
            top = s >= 0
            bottom = s < NT - 1
            u = s + 1
            if bottom and u >= n_enc and dec_w is None:
                dec_w = load_wset(1)
            wtop = enc_w if (top and s < n_enc) else dec_w
            wbot = enc_w if (bottom and u < n_enc) else dec_w
            feedback = bottom and u > n_enc and not teacher

            stage = None
            if bottom and not feedback:
                stage = (stage_override if stage_override is not None
                         else xall[:, u * B:(u + 1) * B])

            gps = [gpool.tile([128, 512], F32, tag="gp", name=f"gp{s}_{j}")
                   for j in range(NCH)]
            ts = tset()
            c_new = statesL.tile([128, 512], F32, tag="c", name=f"c{s}")
            f_new = statesL.tile([128, 512], MDT, tag="hTf", name=f"hTf{s}", bufs=8)
            f_prev = fused

            # PSUM pending-zero granularity: (partition rows of the pass)
            # x (full 2KB bank width).  Exactly one start=True per
            # (bank, row-group) per pair -- the first pass in stream order.
            started = {}
            if not top:
                for j in range(NCH):
                    nc.vector.memset(gps[j][TOP, :], 0.0)
            if not bottom:
                for j in range(NCH):
                    nc.vector.memset(gps[j][BOT, :], 0.0)

            def h1ap(k):
                return f_prev[:, k * B * 2:k * B * 2 + B]

            def h0ap(k):
                return f_prev[:, k * B * 2 + B:(k + 1) * B * 2]

            def emit_bias():
                """Layer-1 bias passes, emitted FIRST: they have no h
                dependency and fill the PE while the previous pair's tail
                (transposes/copies) completes."""
                for j in CO:
                    key = (j, 0)
                    st = not started.get(key, False)
                    started[key] = True
                    nc.tensor.matmul(gps[j][TOP, :], ones64[0:1, :],
                                     wtop["r1"][0:1, j * 512:(j + 1) * 512],
                                     start=st, stop=False,
                                     tile_position=A_POS, skip_group_check=True)

            def emit_streams(with_x):
                """w1i/w1h (A group) zipped with w0/x (B group), all N=512."""
                a_seq, b_seq = [], []
                for j in CO:
                    ws = slice(j * 512, (j + 1) * 512)
                    oT = gps[j][TOP, :]
                    oB = gps[j][BOT, :]
                    if top:
                        for k in (0, 1):
                            a_seq.append((j, oT, h0ap(k),
                                          wtop["w1i"][:, k, ws], False))
                        for k in (0, 1):
                            a_seq.append((j, oT, h1ap(k),
                                          wtop["w1h"][:, k, ws], False))
                        for k in (2, 3):
                            a_seq.append((j, oT, h0ap(k),
                                          wtop["w1i"][:, k, ws], False))
                        for k in (2, 3):
                            a_seq.append((j, oT, h1ap(k),
                                          wtop["w1h"][:, k, ws], k == 3))
                    if bottom:
                        if s >= 0:
                            for k in range(KC):
                                b_seq.append((j, oB, h0ap(k),
                                              wbot["w0"][:, k, ws], False))
                        if with_x:
                            b_seq.append((j, oB, stage[0:2, :],
                                          wbot["r0"][0:2, ws], True))
                order = []
                ia = ib = 0
                while ia < len(a_seq) or ib < len(b_seq):
                    if ia < len(a_seq):
                        order.append(a_seq[ia]); ia += 1
                    if ib < len(b_seq):
                        order.append(b_seq[ib]); ib += 1
                for j, out, lhsT, rhs, sp in order:
                    key = (j, out.base_partition())
                    st = not started.get(key, False)
                    started[key] = True
                    grp = B_POS if out.base_partition() == 64 else A_POS
                    nc.tensor.matmul(out, lhsT, rhs, start=st, stop=sp,
                                     tile_position=grp, skip_group_check=True)

            def emit_xfb():
                """Feedback x-passes: gates0 += (y, 1) @ (Wih0_row, b0)."""
                for j in CO:
                    nc.tensor.matmul(
                        gps[j][BOT, :], dec_stage[0:2, :],
                        wbot["r0"][0:2, j * 512:(j + 1) * 512],
                        start=False, stop=True,
                        tile_position=B_POS, skip_group_check=True)

            def emit_chain_pre(rows=None):
                """Full-width ACT for the early gate chunks (g, i, f)."""
                r = slice(0, 128) if rows is None else rows
                nc.scalar.activation(ts["gsb"][r, :], gps[G_SL][r, :], AF.Tanh)
                nc.scalar.activation(ts["ifsb"][r, 0:512], gps[I_SL][r, :],
                                     AF.Sigmoid)
                nc.scalar.activation(ts["ifsb"][r, 512:1024], gps[F_SL][r, :],
                                     AF.Sigmoid)

            def emit_chain_half(h, rows=None, defer=False):
                """c/o chain for column-half h.  defer=True: queue the
                transposes + f_new copy to run after the NEXT pair's bias
                passes (keeps the PE dense across the pair boundary)."""
                hc = slice(h * HB, (h + 1) * HB)
                r = slice(0, 128) if rows is None else rows
                nc.vector.tensor_mul(ts["t2"][r, hc], ts["ifsb"][r, hc],
                                     ts["gsb"][r, hc])
                nc.vector.tensor_mul(ts["t1"][r, hc],
                                     ts["ifsb"][r, 512 + h * HB:512 + (h + 1) * HB],
                                     c_prev[r, hc])
                nc.vector.tensor_add(c_new[r, hc], ts["t1"][r, hc], ts["t2"][r, hc])
                nc.scalar.activation(ts["tcsb"][r, hc], c_new[r, hc], AF.Tanh)
                for k in (2 * h, 2 * h + 1):
                    ks = slice(k * 128, (k + 1) * 128)
                    nc.scalar.activation(ts["osb"][r, ks], gps[O_SL][r, ks],
                                         AF.Sigmoid)
                    nc.vector.tensor_mul(ts["hsb"][r, ks], ts["osb"][r, ks],
                                         ts["tcsb"][r, ks])

                my_ts, my_f, my_s = ts, f_new, s

                def tr_copy():
                    tp = tpool.tile([128, 256], MDT, tag="tp",
                                    name=f"tp{my_s}_{h}_{r.start}")
                    for k in (2 * h, 2 * h + 1):
                        ks = slice(k * 128, (k + 1) * 128)
                        if rows is None:
                            nc.tensor.transpose(
                                tp[:, (k % 2) * 128:(k % 2 + 1) * 128],
                                my_ts["hsb"][:, ks], identM)
                        else:
                            off = 0 if rows.start == 0 else B
                            nc.tensor.transpose(
                                tp[:, (k % 2) * 128 + off:(k % 2) * 128 + off + B],
                                my_ts["hsb"][r, ks], identM[r, r])
                    if rows is None:
                        nc.vector.tensor_copy(my_f[:, h * 256:(h + 1) * 256], tp)
                    else:
                        off = 0 if rows.start == 0 else B
                        for k in (2 * h, 2 * h + 1):
                            nc.vector.tensor_copy(
                                my_f[:, k * B * 2 + off:k * B * 2 + off + B],
                                tp[:, (k % 2) * 128 + off:(k % 2) * 128 + off + B])

                if defer:
                    pending_tail.append(tr_copy)
                else:
                    tr_copy()

            def emit_head(to_stage):
                """Head matmul y = headW @ h1 + headb (fp32 PSUM), copy to
                out_acc; if to_stage, also round into dec_stage row 0 for the
                next bottom x-pass."""
                d = s - n_enc
                hd = tpool.tile([128, 256], F32, tag="hd", name=f"hd{s}")[0:1, 0:B]
                nc.tensor.matmul(hd, ones64[0:1, 0:1], headb[0:1, :],
                                 start=True, stop=False)
                for k in range(KC):
                    nc.tensor.matmul(hd, headt[:, k:k + 1],
                                     f_new[:, k * B * 2:k * B * 2 + B],
                                     start=False, stop=k == KC - 1)
                if to_stage:
                    nc.vector.tensor_copy(dec_stage[0:1, :], hd)
                nc.vector.tensor_copy(out_acc[0:1, d * B:(d + 1) * B], hd)

            def emit_xfb():
                """Feedback x-passes: gates0 += (y, 1) @ (Wih0_row, b0)."""
                for h in range(2):
                    for j in CO:
                        wsl = slice(j * 512 + h * HB, j * 512 + (h + 1) * HB)
                        nc.tensor.matmul(
                            gps[j][BOT, h * HB:(h + 1) * HB],
                            dec_stage[0:2, :], wbot["r0"][0:2, wsl],
                            start=False, stop=True,
                            tile_position=B_POS, skip_group_check=True)

            def emit_chain(h, rows=None):
                """Cell chain for half h.  rows=None: both cells fused
                (transposes [128,128]); rows=TOP/BOT: single cell
                (transposes [64,128] -> f_new 64-col pieces)."""
                hc = slice(h * HB, (h + 1) * HB)
                r = slice(0, 128) if rows is None else rows
                nc.scalar.activation(ts["gsb"][r, hc], gps[G_SL][r, hc], AF.Tanh)
                nc.scalar.activation(ts["ifsb"][r, h * HB:(h + 1) * HB],
                                     gps[I_SL][r, hc], AF.Sigmoid)
                nc.scalar.activation(ts["ifsb"][r, 512 + h * HB:512 + (h + 1) * HB],
                                     gps[F_SL][r, hc], AF.Sigmoid)
                nc.vector.tensor_mul(ts["t2"][r, hc],
                                     ts["ifsb"][r, h * HB:(h + 1) * HB],
                                     ts["gsb"][r, hc])
                nc.vector.tensor_mul(ts["t1"][r, hc],
                                     ts["ifsb"][r, 512 + h * HB:512 + (h + 1) * HB],
                                     c_prev[r, hc])
                nc.vector.tensor_add(c_new[r, hc], ts["t1"][r, hc], ts["t2"][r, hc])
                nc.scalar.activation(ts["tcsb"][r, hc], c_new[r, hc], AF.Tanh)
                tp = tpool.tile([128, 256], MDT, tag="tp",
                                name=f"tp{s}_{h}_{r.start}")
                for k in (2 * h, 2 * h + 1):
                    ks = slice(k * 128, (k + 1) * 128)
                    nc.scalar.activation(ts["osb"][r, ks], gps[O_SL][r, ks],
                                         AF.Sigmoid)
                    nc.vector.tensor_mul(ts["hsb"][r, ks], ts["osb"][r, ks],
                                         ts["tcsb"][r, ks])
                    if rows is None:
                        nc.tensor.transpose(tp[:, (k % 2) * 128:(k % 2 + 1) * 128],
                                            ts["hsb"][:, ks], identM)
                    else:
                        off = 0 if rows.start == 0 else B
                        idsl = identM[r, r]
                        nc.tensor.transpose(
                            tp[:, (k % 2) * 128 + off:(k % 2) * 128 + off + B],
                            ts["hsb"][r, ks], idsl)
                if rows is None:
                    nc.vector.tensor_copy(f_new[:, h * 256:(h + 1) * 256], tp)
                else:
                    off = 0 if rows.start == 0 else B
                    for k in (2 * h, 2 * h + 1):
                        nc.vector.tensor_copy(
                            f_new[:, k * B * 2 + off:k * B * 2 + off + B],
                            tp[:, (k % 2) * 128 + off:(k % 2) * 128 + off + B])

            if top:
                emit_bias()
            while pending_tail:
                pending_tail.pop(0)()
            if not feedback:
                dfr = can_defer and not (top and s >= n_enc)
                emit_streams(with_x=bottom)
                emit_chain_pre()
                emit_chain_half(0, defer=dfr)
                emit_chain_half(1, defer=dfr)
                if top and s >= n_enc:
                    emit_head(to_stage=False)
            else:
                emit_streams(with_x=False)
                emit_chain_pre(rows=TOP)
                emit_chain_half(0, rows=TOP)
                emit_chain_half(1, rows=TOP)
                emit_head(to_stage=s + 1 < NT)
                emit_xfb()
                emit_chain_pre(rows=BOT)
                emit_chain_half(0, rows=BOT)
                emit_chain_half(1, rows=BOT)

            fused = f_new
            c_prev = c_new

        # ---- schedule all pairs ----
        emit_pair(-1)
        BODY = 8
        nloop = max(0, (n_enc - 2) // BODY)
        s_after_loop = 1 + nloop * BODY
        emit_pair(0)
        while pending_tail:
            pending_tail.pop(0)()
        if nloop > 0:
            from concourse.bass import ds
            with tc.For_i(0, nloop, 1,
                          hint_engines=(mybir.EngineType.PE,)) as iv:
                stage_body = xstage.tile([2, BODY * B], MDT, tag="stage_body")
                nc.sync.dma_start(
                    out=stage_body,
                    in_=xaug[:, ds(2 * B + iv * (BODY * B), BODY * B)])
                for p in range(BODY):
                    emit_pair(1 + p,
                              stage_override=stage_body[:, p * B:(p + 1) * B],
                              can_defer=p < BODY - 1)
                while pending_tail:
                    pending_tail.pop(0)()
        for s in range(s_after_loop, NT):
            emit_pair(s)
        while pending_tail:
            pending_tail.pop(0)()
        nc.sync.dma_start(out=outT[:, :], in_=out_acc)

    nc.compile()
    return nc


# ------------------------------------------------------------------
# host-side packing
# ------------------------------------------------------------------
def _np_dt(mm_dt):
    if mm_dt == "bfloat16":
        import ml_dtypes
        return ml_dtypes.bfloat16
    if mm_dt == "float16":
        return np.float16
    return np.float32


def _pack_weights(inputs, mm_dt="float32r"):
    f32 = np.float32
    ndt = _np_dt(mm_dt)

    def wt_pack(w):  # [G, H] -> [128, KC, G]
        return np.ascontiguousarray(
            np.asarray(w, f32).T.reshape(KC, 128, G).transpose(1, 0, 2)).astype(ndt)

    m = {
        "wt_e0": wt_pack(inputs["enc_Whh0"]),
        "wt_e1i": wt_pack(inputs["enc_Wih1"]),
        "wt_e1h": wt_pack(inputs["enc_Whh1"]),
        "wt_d0": wt_pack(inputs["dec_Whh0"]),
        "wt_d1i": wt_pack(inputs["dec_Wih1"]),
        "wt_d1h": wt_pack(inputs["dec_Whh1"]),
        "rows_e0": np.stack([np.asarray(inputs["enc_Wih0"], f32)[:, 0],
                             np.asarray(inputs["enc_b0"], f32)]).astype(ndt),
        "rows_e1": np.asarray(inputs["enc_b1"], f32)[None, :].astype(ndt),
        "rows_d0": np.stack([np.asarray(inputs["dec_Wih0"], f32)[:, 0],
                             np.asarray(inputs["dec_b0"], f32)]).astype(ndt),
        "rows_d1": np.asarray(inputs["dec_b1"], f32)[None, :].astype(ndt),
        "headt": np.ascontiguousarray(
            np.asarray(inputs["head_W"], f32)[0].reshape(KC, 128).T).astype(ndt),
        "headb": np.full((1, B), float(np.asarray(inputs["head_b"])[0]), ndt),
        "zeros": np.zeros((128, KC * B), ndt),
    }
    return {k: np.ascontiguousarray(v) for k, v in m.items()}


def _pack_x(xc, n_enc=T, mm_dt="float32r", core=0, n_dec=HORIZON):
    """xc [B, T, 1] slice -> xaug [2, (n_enc+1)*B] (row0 = x_t seq, row1 = 1).

    Debug: LSTM_TEACHER=<npz> packs [2, (n_enc+n_dec)*B] with the
    reference outputs as decoder inputs (teacher forcing)."""
    f32 = np.float32
    teacher = os.environ.get("LSTM_TEACHER", "")
    xt = np.asarray(xc, f32)[:, :, 0].T  # [T, B]
    nxa = (n_enc + n_dec) if teacher else (n_enc + 1)
    xa = np.empty((2, nxa * B), f32)
    xa[0, :n_enc * B] = xt[:n_enc].reshape(-1)
    xa[0, n_enc * B:(n_enc + 1) * B] = xt[T - 1]
    xa[1, :] = 1.0
    if teacher:
        y = np.load(teacher)["__out__"]  # [B_FULL, n_dec]
        for d in range(1, n_dec):
            xa[0, (n_enc + d) * B:(n_enc + d + 1) * B] = \
                y[core * B:(core + 1) * B, d - 1]
    return np.ascontiguousarray(xa.astype(_np_dt(mm_dt)))


def _pack_weights2(inputs, mm_dt="float16"):
    """v2 packing: v1 weight set plus W' = outer(dec_Wih0, head_W) and
    rows_dP = dec_b0 + dec_Wih0 * head_b (decoder feedback folded in)."""
    f32 = np.float32
    ndt = _np_dt(mm_dt)

    def wt_pack(w):  # [G, H] -> [128, KC, G]
        return np.ascontiguousarray(
            np.asarray(w, f32).T.reshape(KC, 128, G).transpose(1, 0, 2)).astype(ndt)

    wih0 = np.asarray(inputs["dec_Wih0"], f32)[:, 0]          # [G]
    headW = np.asarray(inputs["head_W"], f32)[0]              # [H]
    headb = float(np.asarray(inputs["head_b"])[0])
    wprime = np.outer(wih0, headW)                            # [G, H]
    m = {
        "wt_e0": wt_pack(inputs["enc_Whh0"]),
        "wt_e1i": wt_pack(inputs["enc_Wih1"]),
        "wt_e1h": wt_pack(inputs["enc_Whh1"]),
        "wt_d0": wt_pack(inputs["dec_Whh0"]),
        "wt_d1i": wt_pack(inputs["dec_Wih1"]),
        "wt_d1h": wt_pack(inputs["dec_Whh1"]),
        "wt_dW": wt_pack(wprime),
        "rows_e0": np.stack([np.asarray(inputs["enc_Wih0"], f32)[:, 0],
                             np.asarray(inputs["enc_b0"], f32)]).astype(ndt),
        "rows_e1": np.asarray(inputs["enc_b1"], f32)[None, :].astype(ndt),
        "rows_d0": np.stack([wih0,
                             np.asarray(inputs["dec_b0"], f32)]).astype(ndt),
        "rows_d1": np.asarray(inputs["dec_b1"], f32)[None, :].astype(ndt),
        "rows_dP": (np.asarray(inputs["dec_b0"], f32)
                    + wih0 * headb)[None, :].astype(ndt),
        "headt": np.ascontiguousarray(
            np.asarray(inputs["head_W"], f32)[0].reshape(KC, 128).T).astype(ndt),
        "headb": np.full((1, B), headb, ndt),
    }
    return {k: np.ascontiguousarray(v) for k, v in m.items()}


def kernel(**inputs):
    global LAST_EXEC_NS, LAST_RESULTS
    from concourse.bass_utils import run_bass_kernel_spmd

    n_enc = int(os.environ.get("LSTM_NENC", T))
    n_dec = int(os.environ.get("LSTM_NDEC", HORIZON))
    mm_dt = os.environ.get("LSTM_MMDT", "float16")
    ver = os.environ.get("LSTM_V", "2")
    key = (n_enc, n_dec, mm_dt, ver)
    if key not in _CACHE:
        _CACHE[key] = (_build2 if ver == "2" else _build)(n_enc, n_dec, mm_dt)
    nc = _CACHE[key]

    shared = (_pack_weights2 if ver == "2" else _pack_weights)(inputs, mm_dt)
    in_maps = []
    for c in range(NCORES):
        m = dict(shared)
        m["xaug"] = _pack_x(inputs["x"][c * B:(c + 1) * B], n_enc, mm_dt,
                            core=c, n_dec=n_dec)
        in_maps.append(m)

    trace = os.environ.get("LSTM_TRACE", "0") == "1"
    t0 = time.time()
    res = run_bass_kernel_spmd(nc, in_maps, list(range(NCORES)), trace=trace)
    wall = time.time() - t0
    LAST_EXEC_NS = res.exec_time_ns
    LAST_RESULTS = res
    if os.environ.get("LSTM_VERBOSE", "0") == "1":
        print(f"[kernel] wall={wall:.2f}s exec_time_ns={res.exec_time_ns}", file=sys.stderr)

    out = np.empty((B_FULL, n_dec), np.float32)
    for c in range(NCORES):
        out[c * B:(c + 1) * B, :] = res.results[c]["outT"].reshape(n_dec, B).T
    return out


if __name__ == "__main__":
    cmd = sys.argv[1] if len(sys.argv) > 1 else "build"
    if cmd == "build":
        ne = int(os.environ.get("LSTM_NENC", "4"))
        nd = int(os.environ.get("LSTM_NDEC", "2"))
        md = os.environ.get("LSTM_MMDT", "float32r")
        t0 = time.time()
        nc = _build(ne, nd, md)
        print(f"build({ne},{nd},{md}) ok in {time.time()-t0:.1f}s")

